# revision 11
# baseline (speedup 1.0000x reference)
"""Dual-score attention kernel for Trainium2 (8 NeuronCores).

Problem: out = softmax((Q_aux K_aux^T * s) * (Q_self K_self^T * s)) @ x
with b=2, n=2048, dim=256, 8 heads of dim 32.

Sharding: 16 (batch, head) units across 8 cores -> each core computes one
batch's pair of heads (zero inter-core communication; output columns are
per-head contiguous).

Per-core dataflow, per 512-wide query block (4 blocks):
  - scores: 4-way row-packed K=32 matmuls (f32r) -> s^T [j, i] in PSUM,
    two j-tiles per 8-bank PSUM pair (aux / self)
  - self-score evacuation PSUM->SBUF fp16 (a single instruction cannot read
    two PSUM operands), product on VectorE -> fp16, exp on ScalarE
    (scale=1/32 fused) -> bf16, one whole-block op
  - p@v: bf16 matmuls, contraction over j; V = [x | 1] so column 256
    accumulates the softmax denominator; final divide happens on host
Projections run once up front (f32r); q rows are partition-swapped via an
SBUF->SBUF DMA (engines cannot cross partitions, DMA can).
"""

import sys

sys.path.insert(0, "/opt/trn_rl_repo")

import numpy as np
import ml_dtypes

import concourse.bacc as bacc
import concourse.mybir as mybir
import concourse.tile as tile
from concourse.bass_utils import run_bass_kernel_spmd

F32 = mybir.dt.float32
F32R = mybir.dt.float32r
BF16 = mybir.dt.bfloat16
FP16 = mybir.dt.float16

B, N, DIM = 2, 2048, 256
HEADS, DH = 8, 32
SCALE2 = 1.0 / DH  # SCALE**2
P = 128
NB = N // 512  # 4 i-blocks of 512
NJ = N // P  # 16 j-tiles of 128
OW = DIM + 1  # 257: out columns per head incl. denominator

_CACHED_NC = {}


def _build_nc(reps=1):
    if reps in _CACHED_NC:
        return _CACHED_NC[reps]

    nc = bacc.Bacc("TRN2", target_bir_lowering=False, debug=False)

    xaf_d = nc.dram_tensor("xaf", [2 * DIM, N], F32R, kind="ExternalInput")
    xv_d = nc.dram_tensor("xv", [N, OW], BF16, kind="ExternalInput")
    w_d = nc.dram_tensor("w", [DIM, 256], F32R, kind="ExternalInput")
    out_d = nc.dram_tensor("out", [N, 2 * OW], F32, kind="ExternalOutput")
    attn0_d = nc.dram_tensor("attn0", [P, 2 * NJ], BF16, kind="ExternalOutput")

    Exp = mybir.ActivationFunctionType.Exp
    Mult = mybir.AluOpType.mult

    with tile.TileContext(nc) as tc:
      for _rep in range(reps):
        with (
            tc.tile_pool(name="const", bufs=1) as const,
            tc.tile_pool(name="svpool", bufs=2) as svpool,
            tc.tile_pool(name="obpool", bufs=2) as obpool,
        ):
            # ---- input staging ----
            # xaf rows: [x.T (256) | attn_feat.T (256)]
            xaf_sb = const.tile([P, 4, N], F32R, tag="xaf")
            xv_sb = const.tile([P, NJ, OW], BF16, tag="xv")
            w_sb = const.tile([P, 2, 256], F32R, tag="w")
            nc.sync.dma_start(w_sb[:], w_d.rearrange("(o p) m -> p o m", p=P))
            nc.sync.dma_start(xaf_sb[:], xaf_d.rearrange("(o p) n -> p o n", p=P))
            nc.sync.dma_start(xv_sb[:], xv_d.rearrange("(j p) d -> p j d", p=P))

            auxT_sb = const.tile([P, N], F32R, tag="auxT")
            selfT_sb = const.tile([P, N], F32R, tag="selfT")
            qT_sb = const.tile([P, N], F32R, tag="qT")
            t_sb = const.tile([P, NJ * 1024], FP16, tag="t")
            e_sb = const.tile([P, NJ * 1024], BF16, tag="e")
            attn0_sb = const.tile([P, 2 * NJ], BF16, tag="attn0")

            # ---- projections ----
            # w[:, 0:128] (aux, x attn_feat): cols [Wk_h0|Wk_h1|Wq_h0|Wq_h1]
            # w[:, 128:256] (self, x x):      cols [Wq_h0|Wq_h1|Wk_h0|Wk_h1]
            # auxT rows:  [k_h0_aux | k_h1_aux | q_h0_aux | q_h1_aux]
            # selfT rows: [q_h0_self | q_h1_self | k_h0_self | k_h1_self]
            # qT rows:    [q_h0_aux | q_h1_aux | q_h0_self | q_h1_self]... no:
            # qT[0:64] <- auxT[64:128] (q_aux), qT[64:128] <- selfT[0:64]
            # giving qT rows [q_h0_aux | q_h1_aux | q_h0_self | q_h1_self],
            # row-aligned with lhsT groups (auxT[0:64] k_aux, selfT[64:128]
            # k_self).
            with tc.tile_pool(name="ppsum", bufs=2, space="PSUM") as ppsum:
                for ib in range(NB):
                    isl = slice(ib * 512, (ib + 1) * 512)
                    pa = ppsum.tile([P, 512], F32, tag="proj", name="pa")
                    pq = ppsum.tile([P, 512], F32, tag="proj", name="pq")
                    for c2 in range(2):
                        nc.tensor.matmul(
                            pa[:], w_sb[:, c2, 0:128], xaf_sb[:, 2 + c2, isl],
                            start=(c2 == 0), stop=(c2 == 1),
                        )
                    for c2 in range(2):
                        nc.tensor.matmul(
                            pq[:], w_sb[:, c2, 128:256], xaf_sb[:, c2, isl],
                            start=(c2 == 0), stop=(c2 == 1),
                        )
                    nc.scalar.copy(auxT_sb[:, isl], pa[:])
                    nc.scalar.copy(selfT_sb[:, isl], pq[:])
                nc.sync.dma_start(qT_sb[0:64, :], auxT_sb[64:128, :])
                nc.sync.dma_start(qT_sb[64:128, :], selfT_sb[0:64, :])

            # ---- per-block: scores+softmax phase, then p@v phase ----
            for ib in range(NB):
                isl = slice(ib * 512, (ib + 1) * 512)

                with tc.tile_pool(name="spsum", bufs=2, space="PSUM") as spsum:
                    for jp in range(NJ // 2):
                        psA = spsum.tile([P, 2048], F32, tag="s", name="psA")
                        psB = spsum.tile([P, 2048], F32, tag="s", name="psB")
                        for half in range(2):
                            jt = 2 * jp + half
                            jsl = slice(jt * P, (jt + 1) * P)
                            for r, dst, src in (
                                (0, psA, auxT_sb), (1, psA, auxT_sb),
                                (2, psB, selfT_sb), (3, psB, selfT_sb),
                            ):
                                nc.tensor.matmul(
                                    dst[:, half * 1024 + (r % 2) * 512 :
                                        half * 1024 + (r % 2) * 512 + 512],
                                    src[32 * r : 32 * r + 32, jsl],
                                    qT_sb[32 * r : 32 * r + 32, isl],
                                    start=True, stop=True,
                                    tile_position=(32 * r, 0),
                                )
                        sv = svpool.tile([P, 2048], FP16, tag="sv")
                        nc.scalar.copy(sv[:], psB[:])
                        nc.vector.tensor_tensor(
                            t_sb[:, jp * 2048 : (jp + 1) * 2048], psA[:], sv[:], Mult
                        )
                # one whole-block exp: [128, 16384] fp16 -> bf16
                nc.scalar.activation(e_sb[:], t_sb[:], Exp, scale=SCALE2)
                if ib == 0:
                    # attn row 0 (query i=0): gather e^T[:, 0] columns; col
                    # layout jt*1024 + hh*512 -> stride-512 slice = (jt, hh)
                    nc.scalar.copy(attn0_sb[:], e_sb[:, 0 : NJ * 1024 : 512])

                with tc.tile_pool(name="vpsum", bufs=1, space="PSUM") as vpsum:
                    po = vpsum.tile([P, 8, 512], F32, tag="o")
                    for it in range(4):
                        for hh in range(2):
                            for jt in range(NJ):
                                c0 = jt * 1024 + hh * 512 + it * P
                                nc.tensor.matmul(
                                    po[:, it * 2 + hh, 0:OW],
                                    e_sb[:, c0 : c0 + P],
                                    xv_sb[:, jt, :],
                                    start=(jt == 0), stop=(jt == NJ - 1),
                                )
                    ob = obpool.tile([P, 4, 2, OW], F32, tag="ob", name="ob")
                    nc.scalar.copy(ob[:], po[:, :, 0:OW])
                    nc.sync.dma_start(
                        out_d[isl, :].rearrange(
                            "(it p) (hh d) -> p it hh d", p=P, d=OW
                        ),
                        ob[:],
                    )

            nc.sync.dma_start(attn0_d[:], attn0_sb[:])

    nc.finalize()
    _CACHED_NC[reps] = nc
    return nc


def _make_in_maps(x, attn_feat, W_qkv, W_qkv_aux):
    x = np.ascontiguousarray(np.asarray(x, dtype=np.float32))
    af = np.ascontiguousarray(np.asarray(attn_feat, dtype=np.float32))
    Wq = np.asarray(W_qkv, dtype=np.float32)
    Wa = np.asarray(W_qkv_aux, dtype=np.float32)

    bf = ml_dtypes.bfloat16
    xaf = [
        np.ascontiguousarray(np.concatenate([x[b].T, af[b].T], axis=0))
        for b in range(B)
    ]
    xv = [
        np.ascontiguousarray(
            np.concatenate([x[b], np.ones((N, 1), np.float32)], axis=1)
        ).astype(bf)
        for b in range(B)
    ]

    in_maps = []
    for core in range(8):
        b, h0 = core // 4, 2 * (core % 4)
        cs = slice(h0 * DH, (h0 + 2) * DH)
        w = np.concatenate(
            [
                Wa[:, 256:512][:, cs],  # k aux
                Wa[:, 0:256][:, cs],  # q aux
                Wq[:, 0:256][:, cs],  # q self
                Wq[:, 256:512][:, cs],  # k self
            ],
            axis=1,
        )
        in_maps.append({"xaf": xaf[b], "xv": xv[b], "w": np.ascontiguousarray(w)})
    return in_maps


def kernel(x, attn_feat, W_qkv, W_qkv_aux, use_cls_tokens=None, **_kw):
    in_maps = _make_in_maps(x, attn_feat, W_qkv, W_qkv_aux)
    nc = _build_nc()
    res = run_bass_kernel_spmd(nc, in_maps, core_ids=list(range(8)))
    results = res.results

    out = np.empty((B, N, HEADS * DIM), np.float32)
    attn_flat0 = np.empty((B, HEADS * N), np.float32)
    for core in range(8):
        b, h0 = core // 4, 2 * (core % 4)
        r = results[core]
        o = np.asarray(r["out"])  # [N, 2*OW]
        for hh in range(2):
            blk = o[:, hh * OW : (hh + 1) * OW]
            out[b, :, (h0 + hh) * DIM : (h0 + hh + 1) * DIM] = (
                blk[:, 0:DIM] / blk[:, DIM : DIM + 1]
            )
        a = np.asarray(r["attn0"], dtype=np.float32)  # [128, 32]; col = jt*2+hh
        for hh in range(2):
            v = a[:, hh::2].T.reshape(N)  # [jt, p] -> j = jt*128 + p
            attn_flat0[b, (h0 + hh) * N : (h0 + hh + 1) * N] = v / v.sum()

    return out, out[:, 0].copy(), attn_flat0


# revision 13
# speedup vs baseline: 1.1788x; 1.1788x over previous
"""Dual-score attention kernel for Trainium2 (8 NeuronCores).

Problem: out = softmax((Q_aux K_aux^T * s) * (Q_self K_self^T * s)) @ x
with b=2, n=2048, dim=256, 8 heads of dim 32.

Sharding: 16 (batch, head) units across 8 cores -> each core computes one
batch's pair of heads (zero inter-core communication; output columns are
per-head contiguous).

Per-core dataflow, per 512-wide query block (4 blocks):
  - scores: 4-way row-packed K=32 matmuls (f32r) -> s^T [j, i] in PSUM,
    two j-tiles per 8-bank PSUM pair (aux / self)
  - self-score evacuation PSUM->SBUF fp16 (a single instruction cannot read
    two PSUM operands), product on VectorE -> fp16, exp on ScalarE
    (scale=1/32 fused) -> bf16, one whole-block op
  - p@v: bf16 matmuls, contraction over j; V = [x | 1] so column 256
    accumulates the softmax denominator; final divide happens on host
Projections run once up front (f32r); q rows are partition-swapped via an
SBUF->SBUF DMA (engines cannot cross partitions, DMA can).
"""

import sys

sys.path.insert(0, "/opt/trn_rl_repo")

import numpy as np
import ml_dtypes

import concourse.bacc as bacc
import concourse.mybir as mybir
import concourse.tile as tile
from concourse.bass_utils import run_bass_kernel_spmd

F32 = mybir.dt.float32
F32R = mybir.dt.float32r
BF16 = mybir.dt.bfloat16
FP16 = mybir.dt.float16

B, N, DIM = 2, 2048, 256
HEADS, DH = 8, 32
SCALE2 = 1.0 / DH  # SCALE**2
P = 128
NB = N // 512  # 4 i-blocks of 512
NJ = N // P  # 16 j-tiles of 128
OW = DIM + 2  # 258: out cols per head incl. denominator (+pad; f32r needs even N)

_CACHED_NC = {}


def _build_nc(reps=1):
    if reps in _CACHED_NC:
        return _CACHED_NC[reps]

    nc = bacc.Bacc("TRN2", target_bir_lowering=False, debug=False)

    xaf_d = nc.dram_tensor("xaf", [2 * DIM, N], F32R, kind="ExternalInput")
    xv_d = nc.dram_tensor("xv", [N, OW], F32R, kind="ExternalInput")
    w_d = nc.dram_tensor("w", [DIM, 256], F32R, kind="ExternalInput")
    out_d = nc.dram_tensor("out", [N, 2 * OW], F32, kind="ExternalOutput")
    attn0_d = nc.dram_tensor("attn0", [P, 2 * NJ], BF16, kind="ExternalOutput")

    Exp = mybir.ActivationFunctionType.Exp
    Mult = mybir.AluOpType.mult

    with tile.TileContext(nc) as tc:
      for _rep in range(reps):
        with (
            tc.tile_pool(name="const", bufs=1) as const,
            tc.tile_pool(name="svpool", bufs=2) as svpool,
            tc.tile_pool(name="obpool", bufs=2) as obpool,
        ):
            # ---- input staging ----
            # xaf rows: [x.T (256) | attn_feat.T (256)]
            xaf_sb = const.tile([P, 4, N], F32R, tag="xaf")
            xv_sb = const.tile([P, NJ, OW], F32R, tag="xv")
            w_sb = const.tile([P, 2, 256], F32R, tag="w")
            nc.sync.dma_start(w_sb[:], w_d.rearrange("(o p) m -> p o m", p=P))
            nc.sync.dma_start(xaf_sb[:], xaf_d.rearrange("(o p) n -> p o n", p=P))
            nc.sync.dma_start(xv_sb[:], xv_d.rearrange("(j p) d -> p j d", p=P))

            auxT_sb = const.tile([P, N], F32R, tag="auxT")
            selfT_sb = const.tile([P, N], F32R, tag="selfT")
            qT_sb = const.tile([P, N], F32R, tag="qT")
            t_sb = const.tile([P, NJ * 512], FP16, tag="t")
            e_sb = const.tile([P, NJ * 1024], F32R, tag="e")
            attn0_sb = const.tile([P, 2 * NJ], BF16, tag="attn0")

            # ---- projections ----
            # w[:, 0:128] (aux, x attn_feat): cols [Wk_h0|Wk_h1|Wq_h0|Wq_h1]
            # w[:, 128:256] (self, x x):      cols [Wq_h0|Wq_h1|Wk_h0|Wk_h1]
            # auxT rows:  [k_h0_aux | k_h1_aux | q_h0_aux | q_h1_aux]
            # selfT rows: [q_h0_self | q_h1_self | k_h0_self | k_h1_self]
            # qT rows:    [q_h0_aux | q_h1_aux | q_h0_self | q_h1_self]... no:
            # qT[0:64] <- auxT[64:128] (q_aux), qT[64:128] <- selfT[0:64]
            # giving qT rows [q_h0_aux | q_h1_aux | q_h0_self | q_h1_self],
            # row-aligned with lhsT groups (auxT[0:64] k_aux, selfT[64:128]
            # k_self).
            with tc.tile_pool(name="ppsum", bufs=2, space="PSUM") as ppsum:
                for ib in range(NB):
                    isl = slice(ib * 512, (ib + 1) * 512)
                    pa = ppsum.tile([P, 512], F32, tag="proj", name="pa")
                    pq = ppsum.tile([P, 512], F32, tag="proj", name="pq")
                    for c2 in range(2):
                        nc.tensor.matmul(
                            pa[:], w_sb[:, c2, 0:128], xaf_sb[:, 2 + c2, isl],
                            start=(c2 == 0), stop=(c2 == 1),
                        )
                    for c2 in range(2):
                        nc.tensor.matmul(
                            pq[:], w_sb[:, c2, 128:256], xaf_sb[:, c2, isl],
                            start=(c2 == 0), stop=(c2 == 1),
                        )
                    nc.scalar.copy(auxT_sb[:, isl], pa[:])
                    nc.scalar.copy(selfT_sb[:, isl], pq[:])
                nc.sync.dma_start(qT_sb[0:64, :], auxT_sb[64:128, :])
                nc.sync.dma_start(qT_sb[64:128, :], selfT_sb[0:64, :])

            # ---- per-block: scores+softmax phase, then p@v phase ----
            for ib in range(NB):
                isl = slice(ib * 512, (ib + 1) * 512)

                with tc.tile_pool(name="spsum", bufs=2, space="PSUM") as spsum:
                    for jp in range(NJ // 2):
                        psA = spsum.tile([P, 2048], F32, tag="s", name="psA")
                        psB = spsum.tile([P, 2048], F32, tag="s", name="psB")
                        for half in range(2):
                            jt = 2 * jp + half
                            jsl = slice(jt * P, (jt + 1) * P)
                            for r, dst, src in (
                                (0, psA, auxT_sb), (1, psA, auxT_sb),
                                (2, psB, selfT_sb), (3, psB, selfT_sb),
                            ):
                                nc.tensor.matmul(
                                    dst[:, half * 1024 + (r % 2) * 512 :
                                        half * 1024 + (r % 2) * 512 + 512],
                                    src[32 * r : 32 * r + 32, jsl],
                                    qT_sb[32 * r : 32 * r + 32, isl],
                                    start=True, stop=True,
                                    tile_position=(32 * r, 0),
                                )
                        sv = svpool.tile([P, 2048], FP16, tag="sv")
                        nc.scalar.copy(sv[:], psB[:])
                        nc.vector.tensor_tensor(
                            t_sb[:, (jp % 4) * 2048 : (jp % 4 + 1) * 2048],
                            psA[:], sv[:], Mult,
                        )
                        if jp % 4 == 3:
                            # half-block exp: [128, 8192] fp16 -> f32r
                            nc.scalar.activation(
                                e_sb[:, (jp // 4) * 8192 : (jp // 4 + 1) * 8192],
                                t_sb[:], Exp, scale=SCALE2,
                            )
                if ib == 0:
                    # attn row 0 (query i=0): gather e^T[:, 0] columns; col
                    # layout jt*1024 + hh*512 -> stride-512 slice = (jt, hh)
                    nc.scalar.copy(attn0_sb[:], e_sb[:, 0 : NJ * 1024 : 512])

                with tc.tile_pool(name="vpsum", bufs=1, space="PSUM") as vpsum:
                    po = vpsum.tile([P, 8, 512], F32, tag="o")
                    for it in range(4):
                        for hh in range(2):
                            for jt in range(NJ):
                                c0 = jt * 1024 + hh * 512 + it * P
                                nc.tensor.matmul(
                                    po[:, it * 2 + hh, 0:OW],
                                    e_sb[:, c0 : c0 + P],
                                    xv_sb[:, jt, :],
                                    start=(jt == 0), stop=(jt == NJ - 1),
                                )
                    ob = obpool.tile([P, 4, 2, OW], F32, tag="ob", name="ob")
                    nc.scalar.copy(ob[:], po[:, :, 0:OW])
                    nc.sync.dma_start(
                        out_d[isl, :].rearrange(
                            "(it p) (hh d) -> p it hh d", p=P, d=OW
                        ),
                        ob[:],
                    )

            nc.sync.dma_start(attn0_d[:], attn0_sb[:])

    nc.finalize()
    _CACHED_NC[reps] = nc
    return nc


def _make_in_maps(x, attn_feat, W_qkv, W_qkv_aux):
    x = np.ascontiguousarray(np.asarray(x, dtype=np.float32))
    af = np.ascontiguousarray(np.asarray(attn_feat, dtype=np.float32))
    Wq = np.asarray(W_qkv, dtype=np.float32)
    Wa = np.asarray(W_qkv_aux, dtype=np.float32)

    bf = ml_dtypes.bfloat16
    xaf = [
        np.ascontiguousarray(np.concatenate([x[b].T, af[b].T], axis=0))
        for b in range(B)
    ]
    xv = [
        np.ascontiguousarray(
            np.concatenate(
                [x[b], np.ones((N, 1), np.float32), np.zeros((N, 1), np.float32)],
                axis=1,
            )
        )
        for b in range(B)
    ]

    in_maps = []
    for core in range(8):
        b, h0 = core // 4, 2 * (core % 4)
        cs = slice(h0 * DH, (h0 + 2) * DH)
        w = np.concatenate(
            [
                Wa[:, 256:512][:, cs],  # k aux
                Wa[:, 0:256][:, cs],  # q aux
                Wq[:, 0:256][:, cs],  # q self
                Wq[:, 256:512][:, cs],  # k self
            ],
            axis=1,
        )
        in_maps.append({"xaf": xaf[b], "xv": xv[b], "w": np.ascontiguousarray(w)})
    return in_maps


def kernel(x, attn_feat, W_qkv, W_qkv_aux, use_cls_tokens=None, **_kw):
    in_maps = _make_in_maps(x, attn_feat, W_qkv, W_qkv_aux)
    nc = _build_nc()
    res = run_bass_kernel_spmd(nc, in_maps, core_ids=list(range(8)))
    results = res.results

    out = np.empty((B, N, HEADS * DIM), np.float32)
    attn_flat0 = np.empty((B, HEADS * N), np.float32)
    for core in range(8):
        b, h0 = core // 4, 2 * (core % 4)
        r = results[core]
        o = np.asarray(r["out"])  # [N, 2*OW]
        for hh in range(2):
            blk = o[:, hh * OW : (hh + 1) * OW]
            out[b, :, (h0 + hh) * DIM : (h0 + hh + 1) * DIM] = (
                blk[:, 0:DIM] / blk[:, DIM : DIM + 1]
            )
        a = np.asarray(r["attn0"], dtype=np.float32)  # [128, 32]; col = jt*2+hh
        for hh in range(2):
            v = a[:, hh::2].T.reshape(N)  # [jt, p] -> j = jt*128 + p
            attn_flat0[b, (h0 + hh) * N : (h0 + hh + 1) * N] = v / v.sum()

    return out, out[:, 0].copy(), attn_flat0


# revision 14
# speedup vs baseline: 1.2350x; 1.0477x over previous
"""Dual-score attention kernel for Trainium2 (8 NeuronCores).

Problem: out = softmax((Q_aux K_aux^T * s) * (Q_self K_self^T * s)) @ x
with b=2, n=2048, dim=256, 8 heads of dim 32.

Sharding: 16 (batch, head) units across 8 cores -> each core computes one
batch's pair of heads (zero inter-core communication; output columns are
per-head contiguous).

Per-core dataflow, per 512-wide query block (4 blocks):
  - scores: 4-way row-packed K=32 matmuls (f32r) -> s^T [j, i] in PSUM,
    two j-tiles per 8-bank PSUM pair (aux / self)
  - self-score evacuation PSUM->SBUF fp16 (a single instruction cannot read
    two PSUM operands), product on VectorE -> fp16, exp on ScalarE
    (scale=1/32 fused) -> bf16, one whole-block op
  - p@v: bf16 matmuls, contraction over j; V = [x | 1] so column 256
    accumulates the softmax denominator; final divide happens on host
Projections run once up front (f32r); q rows are partition-swapped via an
SBUF->SBUF DMA (engines cannot cross partitions, DMA can).
"""

import sys

sys.path.insert(0, "/opt/trn_rl_repo")

import numpy as np
import ml_dtypes

import concourse.bacc as bacc
import concourse.mybir as mybir
import concourse.tile as tile
from concourse.bass_utils import run_bass_kernel_spmd

F32 = mybir.dt.float32
F32R = mybir.dt.float32r
BF16 = mybir.dt.bfloat16
FP16 = mybir.dt.float16

B, N, DIM = 2, 2048, 256
HEADS, DH = 8, 32
SCALE2 = 1.0 / DH  # SCALE**2
P = 128
NB = N // 512  # 4 i-blocks of 512
NJ = N // P  # 16 j-tiles of 128
OW = DIM + 2  # 258: out cols per head incl. denominator (+pad; f32r needs even N)

_CACHED_NC = {}


def _build_nc(reps=1):
    if reps in _CACHED_NC:
        return _CACHED_NC[reps]

    nc = bacc.Bacc("TRN2", target_bir_lowering=False, debug=False)

    xaf_d = nc.dram_tensor("xaf", [2 * DIM, N], F32R, kind="ExternalInput")
    xv_d = nc.dram_tensor("xv", [N, OW], F32R, kind="ExternalInput")
    w_d = nc.dram_tensor("w", [DIM, 256], F32R, kind="ExternalInput")
    out_d = nc.dram_tensor("out", [NB, P, 6, 512], F32, kind="ExternalOutput")
    attn0_d = nc.dram_tensor("attn0", [P, 2 * NJ], BF16, kind="ExternalOutput")

    Exp = mybir.ActivationFunctionType.Exp
    Mult = mybir.AluOpType.mult

    with tile.TileContext(nc) as tc:
      for _rep in range(reps):
        with (
            tc.tile_pool(name="const", bufs=1) as const,
            tc.tile_pool(name="svpool", bufs=2) as svpool,
            tc.tile_pool(name="obpool", bufs=2) as obpool,
        ):
            # ---- input staging ----
            # xaf rows: [x.T (256) | attn_feat.T (256)]
            xaf_sb = const.tile([P, 4, N], F32R, tag="xaf")
            xv_sb = const.tile([P, NJ, OW], F32R, tag="xv")
            w_sb = const.tile([P, 2, 256], F32R, tag="w")
            nc.sync.dma_start(w_sb[:], w_d.rearrange("(o p) m -> p o m", p=P))
            nc.sync.dma_start(xaf_sb[:], xaf_d.rearrange("(o p) n -> p o n", p=P))
            nc.sync.dma_start(xv_sb[:], xv_d.rearrange("(j p) d -> p j d", p=P))

            auxT_sb = const.tile([P, N], F32R, tag="auxT")
            selfT_sb = const.tile([P, N], F32R, tag="selfT")
            qT_sb = const.tile([P, N], F32R, tag="qT")
            t_sb = const.tile([P, NJ * 512], FP16, tag="t")
            e_sb = const.tile([P, NJ * 1024], F32R, tag="e")
            attn0_sb = const.tile([P, 2 * NJ], BF16, tag="attn0")

            # ---- projections ----
            # w[:, 0:128] (aux, x attn_feat): cols [Wk_h0|Wk_h1|Wq_h0|Wq_h1]
            # w[:, 128:256] (self, x x):      cols [Wq_h0|Wq_h1|Wk_h0|Wk_h1]
            # auxT rows:  [k_h0_aux | k_h1_aux | q_h0_aux | q_h1_aux]
            # selfT rows: [q_h0_self | q_h1_self | k_h0_self | k_h1_self]
            # qT rows:    [q_h0_aux | q_h1_aux | q_h0_self | q_h1_self]... no:
            # qT[0:64] <- auxT[64:128] (q_aux), qT[64:128] <- selfT[0:64]
            # giving qT rows [q_h0_aux | q_h1_aux | q_h0_self | q_h1_self],
            # row-aligned with lhsT groups (auxT[0:64] k_aux, selfT[64:128]
            # k_self).
            with tc.tile_pool(name="ppsum", bufs=2, space="PSUM") as ppsum:
                for ib in range(NB):
                    isl = slice(ib * 512, (ib + 1) * 512)
                    pa = ppsum.tile([P, 512], F32, tag="proj", name="pa")
                    pq = ppsum.tile([P, 512], F32, tag="proj", name="pq")
                    for c2 in range(2):
                        nc.tensor.matmul(
                            pa[:], w_sb[:, c2, 0:128], xaf_sb[:, 2 + c2, isl],
                            start=(c2 == 0), stop=(c2 == 1),
                        )
                    for c2 in range(2):
                        nc.tensor.matmul(
                            pq[:], w_sb[:, c2, 128:256], xaf_sb[:, c2, isl],
                            start=(c2 == 0), stop=(c2 == 1),
                        )
                    nc.scalar.copy(auxT_sb[:, isl], pa[:])
                    nc.scalar.copy(selfT_sb[:, isl], pq[:])
                nc.sync.dma_start(qT_sb[0:64, :], auxT_sb[64:128, :])
                nc.sync.dma_start(qT_sb[64:128, :], selfT_sb[0:64, :])

            # ---- per-block: scores+softmax phase, then p@v phase ----
            for ib in range(NB):
                isl = slice(ib * 512, (ib + 1) * 512)

                with tc.tile_pool(name="spsum", bufs=2, space="PSUM") as spsum:
                    for jp in range(NJ // 2):
                        psA = spsum.tile([P, 2048], F32, tag="s", name="psA")
                        psB = spsum.tile([P, 2048], F32, tag="s", name="psB")
                        for half in range(2):
                            jt = 2 * jp + half
                            jsl = slice(jt * P, (jt + 1) * P)
                            for r, dst, src in (
                                (0, psA, auxT_sb), (1, psA, auxT_sb),
                                (2, psB, selfT_sb), (3, psB, selfT_sb),
                            ):
                                nc.tensor.matmul(
                                    dst[:, half * 1024 + (r % 2) * 512 :
                                        half * 1024 + (r % 2) * 512 + 512],
                                    src[32 * r : 32 * r + 32, jsl],
                                    qT_sb[32 * r : 32 * r + 32, isl],
                                    start=True, stop=True,
                                    tile_position=(32 * r, 0),
                                )
                        sv = svpool.tile([P, 2048], FP16, tag="sv")
                        nc.scalar.copy(sv[:], psB[:])
                        nc.vector.tensor_tensor(
                            t_sb[:, (jp % 4) * 2048 : (jp % 4 + 1) * 2048],
                            psA[:], sv[:], Mult,
                        )
                        if jp % 4 == 3:
                            # half-block exp: [128, 8192] fp16 -> f32r
                            nc.scalar.activation(
                                e_sb[:, (jp // 4) * 8192 : (jp // 4 + 1) * 8192],
                                t_sb[:], Exp, scale=SCALE2,
                            )
                if ib == 0:
                    # attn row 0 (query i=0): gather e^T[:, 0] columns; col
                    # layout jt*1024 + hh*512 -> stride-512 slice = (jt, hh)
                    nc.scalar.copy(attn0_sb[:], e_sb[:, 0 : NJ * 1024 : 512])

                with tc.tile_pool(name="vpsum", bufs=1, space="PSUM") as vpsum:
                    # transposed p@v: outT[d, i] = sum_j xv[j, d] * e^T[j, i]
                    # lhsT = xv chunk (natural layout), rhs = e^T slice; one
                    # matmul covers all 512 i of the block -> 2 d-chunks + 1
                    # den-chunk (xv cols 256:258 = [ones|zeros], M=2) per head
                    poT = vpsum.tile([P, 4, 512], F32, tag="oT", name="poT")
                    pden = vpsum.tile([P, 2, 512], F32, tag="oD", name="pden")
                    for hh in range(2):
                        for jt in range(NJ):
                            esl = e_sb[:, jt * 1024 + hh * 512 : jt * 1024 + (hh + 1) * 512]
                            st = (jt == 0)
                            sp = (jt == NJ - 1)
                            for c in range(2):
                                nc.tensor.matmul(
                                    poT[:, hh * 2 + c, :],
                                    xv_sb[:, jt, c * 128 : (c + 1) * 128],
                                    esl, start=st, stop=sp,
                                )
                            nc.tensor.matmul(
                                pden[0:2, hh, :],
                                xv_sb[:, jt, 256:258],
                                esl, start=st, stop=sp,
                            )
                    ob = obpool.tile([P, 6, 512], F32, tag="ob", name="ob")
                    nc.scalar.copy(ob[:, 0:4, :], poT[:])
                    nc.scalar.copy(ob[0:1, 4:6, :], pden[0:1, :, :])
                    nc.sync.dma_start(
                        out_d[ib].rearrange("p s i -> p s i"), ob[:]
                    )

            nc.sync.dma_start(attn0_d[:], attn0_sb[:])

    nc.finalize()
    _CACHED_NC[reps] = nc
    return nc


def _make_in_maps(x, attn_feat, W_qkv, W_qkv_aux):
    x = np.ascontiguousarray(np.asarray(x, dtype=np.float32))
    af = np.ascontiguousarray(np.asarray(attn_feat, dtype=np.float32))
    Wq = np.asarray(W_qkv, dtype=np.float32)
    Wa = np.asarray(W_qkv_aux, dtype=np.float32)

    bf = ml_dtypes.bfloat16
    xaf = [
        np.ascontiguousarray(np.concatenate([x[b].T, af[b].T], axis=0))
        for b in range(B)
    ]
    xv = [
        np.ascontiguousarray(
            np.concatenate(
                [x[b], np.ones((N, 1), np.float32), np.zeros((N, 1), np.float32)],
                axis=1,
            )
        )
        for b in range(B)
    ]

    in_maps = []
    for core in range(8):
        b, h0 = core // 4, 2 * (core % 4)
        cs = slice(h0 * DH, (h0 + 2) * DH)
        w = np.concatenate(
            [
                Wa[:, 256:512][:, cs],  # k aux
                Wa[:, 0:256][:, cs],  # q aux
                Wq[:, 0:256][:, cs],  # q self
                Wq[:, 256:512][:, cs],  # k self
            ],
            axis=1,
        )
        in_maps.append({"xaf": xaf[b], "xv": xv[b], "w": np.ascontiguousarray(w)})
    return in_maps


def kernel(x, attn_feat, W_qkv, W_qkv_aux, use_cls_tokens=None, **_kw):
    in_maps = _make_in_maps(x, attn_feat, W_qkv, W_qkv_aux)
    nc = _build_nc()
    res = run_bass_kernel_spmd(nc, in_maps, core_ids=list(range(8)))
    results = res.results

    out = np.empty((B, N, HEADS * DIM), np.float32)
    attn_flat0 = np.empty((B, HEADS * N), np.float32)
    for core in range(8):
        b, h0 = core // 4, 2 * (core % 4)
        r = results[core]
        o = np.asarray(r["out"])  # [NB, 128, 6, 512] transposed blocks
        for hh in range(2):
            # outT rows d = c*128 + p in slots hh*2+c; den at [0, 4+hh]
            oT = np.concatenate([o[:, :, hh * 2, :], o[:, :, hh * 2 + 1, :]], axis=1)
            den = o[:, 0, 4 + hh, :]  # [NB, 512]
            for ib in range(NB):
                out[b, ib * 512 : (ib + 1) * 512,
                    (h0 + hh) * DIM : (h0 + hh + 1) * DIM] = (
                    oT[ib].T / den[ib][:, None]
                )
        a = np.asarray(r["attn0"], dtype=np.float32)  # [128, 32]; col = jt*2+hh
        for hh in range(2):
            v = a[:, hh::2].T.reshape(N)  # [jt, p] -> j = jt*128 + p
            attn_flat0[b, (h0 + hh) * N : (h0 + hh + 1) * N] = v / v.sum()

    return out, out[:, 0].copy(), attn_flat0


# revision 17
# speedup vs baseline: 1.5831x; 1.2818x over previous
"""Dual-score attention kernel for Trainium2 (8 NeuronCores).

Problem: out = softmax((Q_aux K_aux^T * s) * (Q_self K_self^T * s)) @ x
with b=2, n=2048, dim=256, 8 heads of dim 32.

Sharding: 16 (batch, head) units across 8 cores -> each core computes one
batch's pair of heads (zero inter-core communication; output columns are
per-head contiguous).

Per-core dataflow, per 512-wide query block (4 blocks):
  - scores: 4-way row-packed K=32 matmuls (f32r) -> s^T [j, i] in PSUM,
    two j-tiles per 8-bank PSUM pair (aux / self)
  - self-score evacuation PSUM->SBUF fp16 (a single instruction cannot read
    two PSUM operands), product on VectorE -> fp16, exp on ScalarE
    (scale=1/32 fused) -> bf16, one whole-block op
  - p@v: bf16 matmuls, contraction over j; V = [x | 1] so column 256
    accumulates the softmax denominator; final divide happens on host
Projections run once up front (f32r); q rows are partition-swapped via an
SBUF->SBUF DMA (engines cannot cross partitions, DMA can).
"""

import sys

sys.path.insert(0, "/opt/trn_rl_repo")

import numpy as np
import ml_dtypes

import concourse.bacc as bacc
import concourse.mybir as mybir
import concourse.tile as tile
from concourse.bass_utils import run_bass_kernel_spmd

F32 = mybir.dt.float32
F32R = mybir.dt.float32r
BF16 = mybir.dt.bfloat16
FP16 = mybir.dt.float16

B, N, DIM = 2, 2048, 256
HEADS, DH = 8, 32
SCALE2 = 1.0 / DH  # SCALE**2
P = 128
NB = N // 512  # 4 i-blocks of 512
NJ = N // P  # 16 j-tiles of 128
OW = DIM + 2  # 258: out cols per head incl. denominator (+pad; f32r needs even N)

_CACHED_NC = {}


def _build_nc(reps=1):
    if reps in _CACHED_NC:
        return _CACHED_NC[reps]

    nc = bacc.Bacc("TRN2", target_bir_lowering=False, debug=False)

    xaf_d = nc.dram_tensor("xaf", [2 * DIM, N], F32R, kind="ExternalInput")
    xv_d = nc.dram_tensor("xv", [N, OW], F32R, kind="ExternalInput")
    w_d = nc.dram_tensor("w", [DIM, 256], F32R, kind="ExternalInput")
    out_d = nc.dram_tensor("out", [NB, P, 6, 512], F32, kind="ExternalOutput")
    attn0_d = nc.dram_tensor("attn0", [P, 2 * NJ], BF16, kind="ExternalOutput")

    Exp = mybir.ActivationFunctionType.Exp
    Mult = mybir.AluOpType.mult

    with tile.TileContext(nc) as tc:
      for _rep in range(reps):
        with (
            tc.tile_pool(name="const", bufs=1) as const,
            tc.tile_pool(name="svpool", bufs=2) as svpool,
            tc.tile_pool(name="obpool", bufs=2) as obpool,
            tc.tile_pool(name="drpool", bufs=2) as drpool,
        ):
            # ---- input staging ----
            # xaf rows: [x.T (256) | attn_feat.T (256)]
            xaf_sb = const.tile([P, 4, N], F32R, tag="xaf")
            xv_sb = const.tile([P, NJ, OW], F32R, tag="xv")
            w_sb = const.tile([P, 2, 256], F32R, tag="w")
            nc.sync.dma_start(w_sb[:], w_d.rearrange("(o p) m -> p o m", p=P))
            nc.sync.dma_start(xaf_sb[:], xaf_d.rearrange("(o p) n -> p o n", p=P))
            nc.sync.dma_start(xv_sb[:], xv_d.rearrange("(j p) d -> p j d", p=P))

            auxT_sb = const.tile([P, N], F32R, tag="auxT")
            selfT_sb = const.tile([P, N], F32R, tag="selfT")
            qT_sb = const.tile([P, N], F32R, tag="qT")
            t_sb = const.tile([P, NJ * 512], FP16, tag="t")
            e_sb = const.tile([P, NJ * 1024], F32R, tag="e")
            attn0_sb = const.tile([P, 2 * NJ], BF16, tag="attn0")

            # ---- projections ----
            # w[:, 0:128] (aux, x attn_feat): cols [Wk_h0|Wk_h1|Wq_h0|Wq_h1]
            # w[:, 128:256] (self, x x):      cols [Wq_h0|Wq_h1|Wk_h0|Wk_h1]
            # auxT rows:  [k_h0_aux | k_h1_aux | q_h0_aux | q_h1_aux]
            # selfT rows: [q_h0_self | q_h1_self | k_h0_self | k_h1_self]
            # qT rows:    [q_h0_aux | q_h1_aux | q_h0_self | q_h1_self]... no:
            # qT[0:64] <- auxT[64:128] (q_aux), qT[64:128] <- selfT[0:64]
            # giving qT rows [q_h0_aux | q_h1_aux | q_h0_self | q_h1_self],
            # row-aligned with lhsT groups (auxT[0:64] k_aux, selfT[64:128]
            # k_self).
            with tc.tile_pool(name="ppsum", bufs=2, space="PSUM") as ppsum:
                for ib in range(NB):
                    isl = slice(ib * 512, (ib + 1) * 512)
                    pa = ppsum.tile([P, 512], F32, tag="proj", name="pa")
                    pq = ppsum.tile([P, 512], F32, tag="proj", name="pq")
                    for c2 in range(2):
                        nc.tensor.matmul(
                            pa[:], w_sb[:, c2, 0:128], xaf_sb[:, 2 + c2, isl],
                            start=(c2 == 0), stop=(c2 == 1),
                        )
                    for c2 in range(2):
                        nc.tensor.matmul(
                            pq[:], w_sb[:, c2, 128:256], xaf_sb[:, c2, isl],
                            start=(c2 == 0), stop=(c2 == 1),
                        )
                    nc.scalar.copy(auxT_sb[:, isl], pa[:])
                    nc.scalar.copy(selfT_sb[:, isl], pq[:])
                nc.sync.dma_start(qT_sb[0:64, :], auxT_sb[64:128, :])
                nc.sync.dma_start(qT_sb[64:128, :], selfT_sb[0:64, :])

            # ---- per-block: scores+softmax phase, then p@v phase ----
            for ib in range(NB):
                isl = slice(ib * 512, (ib + 1) * 512)

                with tc.tile_pool(name="spsum", bufs=2, space="PSUM") as spsum:
                    for jp in range(NJ // 2):
                        psA = spsum.tile([P, 2048], F32, tag="s", name="psA")
                        psB = spsum.tile([P, 2048], F32, tag="s", name="psB")
                        for half in range(2):
                            jt = 2 * jp + half
                            jsl = slice(jt * P, (jt + 1) * P)
                            for r, dst, src in (
                                (0, psA, auxT_sb), (1, psA, auxT_sb),
                                (2, psB, selfT_sb), (3, psB, selfT_sb),
                            ):
                                nc.tensor.matmul(
                                    dst[:, half * 1024 + (r % 2) * 512 :
                                        half * 1024 + (r % 2) * 512 + 512],
                                    src[32 * r : 32 * r + 32, jsl],
                                    qT_sb[32 * r : 32 * r + 32, isl],
                                    start=True, stop=True,
                                    tile_position=(32 * r, 0),
                                )
                        sv = svpool.tile([P, 2048], FP16, tag="sv")
                        nc.scalar.copy(sv[:], psB[:])
                        nc.vector.tensor_tensor(
                            t_sb[:, (jp % 4) * 2048 : (jp % 4 + 1) * 2048],
                            psA[:], sv[:], Mult,
                        )
                        if jp % 4 == 3:
                            # half-block exp: [128, 8192] fp16 -> f32r
                            nc.scalar.activation(
                                e_sb[:, (jp // 4) * 8192 : (jp // 4 + 1) * 8192],
                                t_sb[:], Exp, scale=SCALE2,
                            )
                if ib == 0:
                    # attn row 0 (query i=0): gather e^T[:, 0] columns; col
                    # layout jt*1024 + hh*512 -> stride-512 slice = (jt, hh)
                    nc.scalar.copy(attn0_sb[:], e_sb[:, 0 : NJ * 1024 : 512])

                with tc.tile_pool(name="vpsum", bufs=1, space="PSUM") as vpsum:
                    # transposed p@v: outT[d, i] = sum_j xv[j, d] * e^T[j, i]
                    # lhsT = xv chunk (natural layout), rhs = e^T slice; one
                    # matmul covers all 512 i of the block -> 2 d-chunks + 1
                    # den-chunk (xv cols 256:258 = [ones|zeros], M=2) per head
                    poT = vpsum.tile([P, 4, 512], F32, tag="oT", name="poT")
                    pden = vpsum.tile([P, 2, 512], F32, tag="oD", name="pden")
                    e_view = e_sb.rearrange("p (jt c) -> p jt c", c=1024)
                    for hh in range(2):
                        for jt in range(NJ):
                            esl = e_sb[:, jt * 1024 + hh * 512 : jt * 1024 + (hh + 1) * 512]
                            st = (jt == 0)
                            sp = (jt == NJ - 1)
                            for c in range(2):
                                nc.tensor.matmul(
                                    poT[:, hh * 2 + c, :],
                                    xv_sb[:, jt, c * 128 : (c + 1) * 128],
                                    esl, start=st, stop=sp,
                                )
                        # den[i] = sum_j e^T[j,i]: DVE-reduce over the 16
                        # j-tiles (free axis), then one M=2 ones-matmul
                        # collapses the 128 partitions
                        dr = drpool.tile([P, 512], F32R, tag="dr", name="dr")
                        with nc.allow_low_precision(
                            reason="f32r is bit-identical f32 storage"
                        ):
                            nc.vector.reduce_sum(
                                dr[:],
                                e_view[:, :, hh * 512 : (hh + 1) * 512].rearrange(
                                    "p jt i -> p i jt"
                                ),
                                axis=mybir.AxisListType.X,
                            )
                        nc.tensor.matmul(
                            pden[0:2, hh, :], xv_sb[:, 0, 256:258], dr[:],
                            start=True, stop=True,
                        )
                    ob = obpool.tile([P, 6, 512], F32, tag="ob", name="ob")
                    nc.scalar.copy(ob[:, 0:4, :], poT[:])
                    nc.scalar.copy(ob[0:1, 4:6, :], pden[0:1, :, :])
                    nc.sync.dma_start(
                        out_d[ib].rearrange("p s i -> p s i"), ob[:]
                    )

            nc.sync.dma_start(attn0_d[:], attn0_sb[:])

    nc.finalize()
    _CACHED_NC[reps] = nc
    return nc


def _make_in_maps(x, attn_feat, W_qkv, W_qkv_aux):
    x = np.ascontiguousarray(np.asarray(x, dtype=np.float32))
    af = np.ascontiguousarray(np.asarray(attn_feat, dtype=np.float32))
    Wq = np.asarray(W_qkv, dtype=np.float32)
    Wa = np.asarray(W_qkv_aux, dtype=np.float32)

    bf = ml_dtypes.bfloat16
    xaf = [
        np.ascontiguousarray(np.concatenate([x[b].T, af[b].T], axis=0))
        for b in range(B)
    ]
    xv = [
        np.ascontiguousarray(
            np.concatenate(
                [x[b], np.ones((N, 1), np.float32), np.zeros((N, 1), np.float32)],
                axis=1,
            )
        )
        for b in range(B)
    ]

    in_maps = []
    for core in range(8):
        b, h0 = core // 4, 2 * (core % 4)
        cs = slice(h0 * DH, (h0 + 2) * DH)
        w = np.concatenate(
            [
                Wa[:, 256:512][:, cs],  # k aux
                Wa[:, 0:256][:, cs],  # q aux
                Wq[:, 0:256][:, cs],  # q self
                Wq[:, 256:512][:, cs],  # k self
            ],
            axis=1,
        )
        in_maps.append({"xaf": xaf[b], "xv": xv[b], "w": np.ascontiguousarray(w)})
    return in_maps


def kernel(x, attn_feat, W_qkv, W_qkv_aux, use_cls_tokens=None, **_kw):
    in_maps = _make_in_maps(x, attn_feat, W_qkv, W_qkv_aux)
    nc = _build_nc()
    res = run_bass_kernel_spmd(nc, in_maps, core_ids=list(range(8)))
    results = res.results

    out = np.empty((B, N, HEADS * DIM), np.float32)
    attn_flat0 = np.empty((B, HEADS * N), np.float32)
    for core in range(8):
        b, h0 = core // 4, 2 * (core % 4)
        r = results[core]
        o = np.asarray(r["out"])  # [NB, 128, 6, 512] transposed blocks
        for hh in range(2):
            # outT rows d = c*128 + p in slots hh*2+c; den at [0, 4+hh]
            oT = np.concatenate([o[:, :, hh * 2, :], o[:, :, hh * 2 + 1, :]], axis=1)
            den = o[:, 0, 4 + hh, :]  # [NB, 512]
            for ib in range(NB):
                out[b, ib * 512 : (ib + 1) * 512,
                    (h0 + hh) * DIM : (h0 + hh + 1) * DIM] = (
                    oT[ib].T / den[ib][:, None]
                )
        a = np.asarray(r["attn0"], dtype=np.float32)  # [128, 32]; col = jt*2+hh
        for hh in range(2):
            v = a[:, hh::2].T.reshape(N)  # [jt, p] -> j = jt*128 + p
            attn_flat0[b, (h0 + hh) * N : (h0 + hh + 1) * N] = v / v.sum()

    return out, out[:, 0].copy(), attn_flat0
